# revision 2
# baseline (speedup 1.0000x reference)
"""BibdLinear Trainium2 kernel: out = input @ (weight * mask).T

Shapes (hardcoded): input [8192, 4096] f32, weight [4096, 4096] f32,
mask [4096, 4096] f32 -> out [8192, 4096] f32.

Sharding (batch-parallel x column-parallel, 8 cores):
  2 batch shards x 4 output-feature shards. Core c handles batch rows
  [(c//4)*4096, +4096) and output features [(c%4)*1024, +1024). The host
  pre-masks the weight (w*mask, a mask-only transform) and ships per-core
  contraction-major bf16 slices; the device runs a pure GEMM and the host
  concatenates the 8 output slices.

Per-core device program (Bass/Tile):
  - inputs: xT [4096, 4096] bf16, wmT [4096, 1024] bf16 (pre-masked).
    bf16 on both operands measures 2.3e-3 rms vs the f32 reference
    (the PE multiplies bf16 exactly in FP22 and accumulates FP32).
  - GEMM: per 256-row batch block, accumulate over 32 k-tiles into PSUM;
    lhsT = x k-tile [128,128] (stationary), rhs = masked-weight chunk
    [128, NF] (moving). PSUM is double-buffered across batch blocks
    (2 x B_SUB x OC tiles = 16KB/partition) so evictions never stall the
    tensor engine.
  - no mask phase on device: weight strips (8MB bf16) stream on the ACT
    queue while block 0 computes; x tiles prefetch on the SP queue;
    PSUM->SBUF evictions on DVE; output stores on the ACT queue.
"""

import numpy as np
import ml_dtypes

import concourse.mybir as mybir
import concourse.tile as tile
from concourse import bacc
from concourse.bass_utils import run_bass_kernel_spmd

BATCH, IN_F, OUT_F = 8192, 4096, 4096
B_S, O_S = 2, 4                      # batch shards x out-feature shards
B, OF = BATCH // B_S, OUT_F // O_S   # 4096, 1024 per core
N_CORES = 8

NB = 256   # batch block width (2 subtiles of 128)
NF = 512   # moving (feature) chunk width per matmul

F32 = mybir.dt.float32
BF16 = mybir.dt.bfloat16

_NC_CACHE = {}


def _build_nc(iters=1, nf=NF, nb=NB, x_bufs=10, out_bufs=4):
    K = IN_F
    KO = K // 128          # 32 contraction tiles
    B_SUB = nb // 128      # batch subtiles per block
    OC = OF // nf          # feature chunks
    NBLK = B // nb         # batch blocks

    nc = bacc.Bacc(None, target_bir_lowering=False)

    xT = nc.dram_tensor("xT", [K, B], BF16, kind="ExternalInput")
    wT = nc.dram_tensor("wT", [K, OF], BF16, kind="ExternalInput")
    out = nc.dram_tensor("out", [B, OF], F32, kind="ExternalOutput")

    xT3 = xT.rearrange("(ko p) b -> ko p b", p=128)
    wT3 = wT.rearrange("(ko p) o -> ko p o", p=128)

    with tile.TileContext(nc) as tc:
        with (
            tc.tile_pool(name="wpool", bufs=1) as wpool,
            tc.tile_pool(name="xpool", bufs=x_bufs) as xpool,
            tc.tile_pool(name="opool", bufs=out_bufs) as opool,
            tc.tile_pool(name="psum", bufs=2, space="PSUM") as psum_pool,
        ):
            def body(_i=None):
                mw = [None] * KO

                for bb in range(NBLK):
                    psums = [
                        psum_pool.tile([128, nf], F32, tag=f"ps{i}",
                                       name=f"ps{i}_{bb}")
                        for i in range(B_SUB * OC)
                    ]
                    for k in range(KO):
                        if bb == 0:
                            mw[k] = wpool.tile([128, OF], BF16, tag=f"mw{k}",
                                               name=f"mw{k}")
                            nc.scalar.dma_start(mw[k], wT3[k])
                        xt = xpool.tile([128, nb], BF16, tag="xt",
                                        name=f"xt{bb}_{k}")
                        nc.sync.dma_start(xt, xT3[k, :, bb * nb:(bb + 1) * nb])
                        for bs in range(B_SUB):
                            lhsT = xt[:, bs * 128:(bs + 1) * 128]
                            for oc in range(OC):
                                nc.tensor.matmul(
                                    psums[bs * OC + oc], lhsT,
                                    mw[k][:, oc * nf:(oc + 1) * nf],
                                    start=(k == 0), stop=(k == KO - 1),
                                )
                    for bs in range(B_SUB):
                        ot = opool.tile([128, OF], F32, tag="ot",
                                        name=f"ot{bb}_{bs}")
                        for oc in range(OC):
                            nc.vector.tensor_copy(
                                ot[:, oc * nf:(oc + 1) * nf],
                                psums[bs * OC + oc],
                            )
                        nc.scalar.dma_start(
                            out[bb * nb + bs * 128: bb * nb + (bs + 1) * 128, :],
                            ot,
                        )

            if iters == 1:
                body()
            else:
                with tc.For_i(0, iters, hint_engines=tuple(mybir.ALL_ENGINES)) as i:
                    body(i)

    nc.compile()
    return nc


def _get_nc():
    if "nc" not in _NC_CACHE:
        _NC_CACHE["nc"] = _build_nc()
    return _NC_CACHE["nc"]


def shard_inputs(input, weight, mask):
    """Host-side sharding/layout: per-core contraction-major bf16 slices,
    weight pre-masked on host (mask-dependent transform only)."""
    bf = ml_dtypes.bfloat16
    x16 = np.asarray(input, dtype=np.float32).astype(bf)
    wm = (np.asarray(weight, dtype=np.float32)
          * np.asarray(mask, dtype=np.float32)).astype(bf)
    xT = x16.T   # [K, BATCH] view
    x_shards = [np.ascontiguousarray(xT[:, s * B:(s + 1) * B])
                for s in range(B_S)]
    w_shards = [np.ascontiguousarray(wm[s * OF:(s + 1) * OF, :].T)
                for s in range(O_S)]
    in_maps = []
    for c in range(N_CORES):
        in_maps.append({
            "xT": x_shards[c // O_S],
            "wT": w_shards[c % O_S],
        })
    return in_maps


def gather_output(results):
    outp = np.empty((BATCH, OUT_F), np.float32)
    for c in range(N_CORES):
        b0 = (c // O_S) * B
        o0 = (c % O_S) * OF
        outp[b0:b0 + B, o0:o0 + OF] = results[c]["out"]
    return outp


def kernel(input, weight, mask):
    in_maps = shard_inputs(input, weight, mask)
    res = run_bass_kernel_spmd(_get_nc(), in_maps, core_ids=list(range(N_CORES)))
    return gather_output(res.results)


# revision 11
# speedup vs baseline: 2.7085x; 2.7085x over previous
"""BibdLinear Trainium2 kernel: out = input @ (weight * mask).T

Shapes (hardcoded): input [8192, 4096] f32, weight [4096, 4096] f32,
mask [4096, 4096] f32 -> out [8192, 4096] f32.

Sharding (batch-parallel x column-parallel, 8 cores):
  2 batch shards x 4 output-feature shards. Core c handles batch rows
  [(c//4)*4096, +4096) and output features [(c%4)*1024, +1024). The host
  pre-masks the weight (w*mask, a mask-only transform) and ships per-core
  contraction-major bf16 slices; the device runs a pure GEMM and the host
  concatenates the 8 output slices.

Per-core device program (Bass/Tile):
  - inputs: xT [4096, 4096] bf16, wmT [4096, 1024] bf16 (pre-masked).
    bf16 on both operands measures 2.3e-3 rms vs the f32 reference
    (the PE multiplies bf16 exactly in FP22 and accumulates FP32).
  - GEMM: per 256-row batch block, accumulate over 32 k-tiles into PSUM;
    lhsT = x k-tile [128,128] (stationary), rhs = masked-weight chunk
    [128, 512] (moving, N=512 = one PSUM bank). PSUM pool bufs=2 double-
    buffers blocks (8 banks total) so evictions never stall the PE.
  - startup: the first two blocks run with k outermost (prologue), so
    each weight strip feeds 8 matmuls the moment it lands — the PE stays
    fed while the 8MB weight load is still in flight on the ACT queue.
  - x tiles span two blocks ([128, 512] per k, 32 tags x 2 bufs on the
    SP queue) halving x DMA count; PSUM->SBUF evictions on DVE; output
    stores on the ACT queue. Measured 459.7us/iter steady (For_i slope,
    see test.py) vs a 441us pure-streaming model for the 2048 N=512
    matmuls; the 497.5us baseline used f32r + on-device masking.
"""

import numpy as np
import ml_dtypes

import concourse.mybir as mybir
import concourse.tile as tile
from concourse import bacc
from concourse.bass_utils import run_bass_kernel_spmd

BATCH, IN_F, OUT_F = 8192, 4096, 4096
B_S, O_S = 2, 4                      # batch shards x out-feature shards
B, OF = BATCH // B_S, OUT_F // O_S   # 4096, 1024 per core
N_CORES = 8

NB = 256   # batch block width (2 subtiles of 128)
NF = 512   # moving (feature) chunk width per matmul

F32 = mybir.dt.float32
BF16 = mybir.dt.bfloat16

_NC_CACHE = {}


def _build_nc(iters=1, nf=NF, nb=NB, out_bufs=4, g_pro=2,
              skip_dma=False, xspan=2, fi_stagger=False):
    K = IN_F
    KO = K // 128          # 32 contraction tiles
    B_SUB = nb // 128      # batch subtiles per block
    OC = OF // nf          # feature chunks
    NBLK = B // nb         # batch blocks
    NPS = B_SUB * OC       # psum tiles per block
    ps_bufs = 2 if NPS <= 4 else 1   # double-buffer blocks when banks allow
    assert not (g_pro and ps_bufs < 2), "prologue needs 2 live psum blocks"

    nc = bacc.Bacc(None, target_bir_lowering=False)

    xT = nc.dram_tensor("xT", [K, B], BF16, kind="ExternalInput")
    wT = nc.dram_tensor("wT", [K, OF], BF16, kind="ExternalInput")
    out = nc.dram_tensor("out", [B, OF], F32, kind="ExternalOutput")

    xT3 = xT.rearrange("(ko p) b -> ko p b", p=128)
    wT3 = wT.rearrange("(ko p) o -> ko p o", p=128)

    with tile.TileContext(nc) as tc:
        with (
            tc.tile_pool(name="wpool", bufs=1) as wpool,
            tc.tile_pool(name="xpool", bufs=2) as xpool,
            tc.tile_pool(name="opool", bufs=out_bufs) as opool,
            tc.tile_pool(name="psum", bufs=ps_bufs, space="PSUM") as psum_pool,
        ):
            def body(_i=None):
                mw = [None] * KO

                # pool bufs=2 rotates two buffers per tag: block bb+1
                # accumulates into the spare buffer while bb's tiles are
                # still being evicted, so the tensor engine never waits.
                def ps_tiles(bb):
                    return [
                        psum_pool.tile([128, nf], F32, tag=f"ps{i}",
                                       name=f"ps{i}_{bb}")
                        for i in range(NPS)
                    ]

                xts = {}

                def x_tile(bb, k):
                    bbp = bb // xspan
                    if (bbp, k) not in xts:
                        xt = xpool.tile([128, nb * xspan], BF16,
                                        tag=f"xt{k % 32}",
                                        name=f"xt{bbp}_{k}")
                        if not skip_dma:
                            nc.sync.dma_start(
                                xt, xT3[k, :, bbp * nb * xspan:
                                        (bbp + 1) * nb * xspan])
                        xts[(bbp, k)] = xt
                    off = (bb % xspan) * nb
                    return xts[(bbp, k)][:, off:off + nb]

                def mm(psums, xt, k, bb):
                    for bs in range(B_SUB):
                        lhsT = xt[:, bs * 128:(bs + 1) * 128]
                        for oc in range(OC):
                            nc.tensor.matmul(
                                psums[bs * OC + oc], lhsT,
                                mw[k][:, oc * nf:(oc + 1) * nf],
                                start=(k == 0), stop=(k == KO - 1),
                            )

                def evict(psums, bb):
                    for bs in range(B_SUB):
                        ot = opool.tile([128, OF], F32, tag="ot",
                                        name=f"ot{bb}_{bs}")
                        for oc in range(OC):
                            nc.vector.tensor_copy(
                                ot[:, oc * nf:(oc + 1) * nf],
                                psums[bs * OC + oc],
                            )
                        nc.scalar.dma_start(
                            out[bb * nb + bs * 128: bb * nb + (bs + 1) * 128, :],
                            ot,
                        )

                # prologue: first g_pro blocks with k outermost, so each
                # weight strip (streaming in on the ACT queue) feeds
                # g_pro*B_SUB*OC matmuls on arrival — the tensor engine
                # stays fed even while the 8MB weight load is in flight.
                g = min(g_pro, NBLK) if g_pro else 0
                if g:
                    pro_ps = [ps_tiles(bb) for bb in range(g)]
                    for k in range(KO):
                        mw[k] = wpool.tile([128, OF], BF16, tag=f"mw{k}",
                                           name=f"mw{k}")
                        if not skip_dma:
                            nc.scalar.dma_start(mw[k], wT3[k])
                        for bb in range(g):
                            mm(pro_ps[bb], x_tile(bb, k), k, bb)
                    for bb in range(g):
                        evict(pro_ps[bb], bb)

                for bb in range(g, NBLK):
                    psums = ps_tiles(bb)
                    for k in range(KO):
                        if not g and bb == 0:
                            mw[k] = wpool.tile([128, OF], BF16, tag=f"mw{k}",
                                               name=f"mw{k}")
                            if not skip_dma:
                                nc.scalar.dma_start(mw[k], wT3[k])
                        mm(psums, x_tile(bb, k), k, bb)
                    evict(psums, bb)

            if iters == 1:
                body()
            else:
                with tc.For_i(0, iters, hint_engines=tuple(mybir.ALL_ENGINES),
                              staggered_reset=fi_stagger) as i:
                    body(i)

    nc.compile()
    return nc


def _get_nc():
    if "nc" not in _NC_CACHE:
        _NC_CACHE["nc"] = _build_nc()
    return _NC_CACHE["nc"]


def shard_inputs(input, weight, mask):
    """Host-side sharding/layout: per-core contraction-major bf16 slices,
    weight pre-masked on host (mask-dependent transform only)."""
    bf = ml_dtypes.bfloat16
    x16 = np.asarray(input, dtype=np.float32).astype(bf)
    wm = (np.asarray(weight, dtype=np.float32)
          * np.asarray(mask, dtype=np.float32)).astype(bf)
    xT = x16.T   # [K, BATCH] view
    x_shards = [np.ascontiguousarray(xT[:, s * B:(s + 1) * B])
                for s in range(B_S)]
    w_shards = [np.ascontiguousarray(wm[s * OF:(s + 1) * OF, :].T)
                for s in range(O_S)]
    in_maps = []
    for c in range(N_CORES):
        in_maps.append({
            "xT": x_shards[c // O_S],
            "wT": w_shards[c % O_S],
        })
    return in_maps


def gather_output(results):
    outp = np.empty((BATCH, OUT_F), np.float32)
    for c in range(N_CORES):
        b0 = (c // O_S) * B
        o0 = (c % O_S) * OF
        outp[b0:b0 + B, o0:o0 + OF] = results[c]["out"]
    return outp


def kernel(input, weight, mask):
    in_maps = shard_inputs(input, weight, mask)
    res = run_bass_kernel_spmd(_get_nc(), in_maps, core_ids=list(range(N_CORES)))
    return gather_output(res.results)
